# revision 39
# baseline (speedup 1.0000x reference)
"""Trainium2 Bass kernel for 2D attention with relative-position augmentation.

Problem shapes (hardcoded): inputs [8, 32, 32, 768] fp32 (q|k|v packed on the
channel axis, 256 each), key_rel_w/key_rel_h [63, 32] fp32.
Output: [8, 32, 32, 256] fp32.

Sharding: data-parallel over batch - core b gets batch b (8 cores, no
collectives needed).

Per-core math (N = 32*32 = 1024 tokens, 8 heads, head dim 32):
  L[n, m] = Q[n].K[m] + qdw[n, y2(m)-y(n)+31] + qdh[n, x2(m)-x(n)+31]
  out[n]  = softmax_m(L[n, :] / sqrt(32)) @ V
where qdw = Q @ key_rel_w^T, qdh = Q @ key_rel_h^T and n=(x,y), m=(x2,y2).

Design notes:
  * Inputs are marshalled on the host (pure layout/dtype transforms - all the
    math runs on device): Q^T/K^T head-major bf16 images, V packed to the SBUF
    partition layout with a ones column appended, the rel tables transposed,
    and the constant one-hot selector rows. This avoids the slow strided
    fp32->bf16 cast DMAs and xbar transpose loads of earlier revisions.
  * Logits are computed transposed (L^T: m on partitions) with the two
    relative-position terms folded into the same matmul by extending the
    contraction dim to 96:
      lhsT rows  0-31: K^T          rhs rows  0-31: Q^T
      lhsT rows 32-63: [y2(m)==y']  rhs rows 32-63: Bw[y',n]=qdw^T[y'-y(n)+31,n]
      lhsT rows 64-95: [x2(m)==x']  rhs rows 64-95: Bh[x',n]=qdh^T[x'-x(n)+31,n]
    B rows are built with one matmul per shift value (stationary = shifted
    free-slice of the rel-table tile, all 8 heads per rhs), then one strided
    PSUM->SBUF copy per group of 8 shifts.
  * Softmax skips max-subtraction; 1/sqrt(32) is folded into the Exp scale.
    exp is split between ScalarE (exact, table-based) and VectorE (a 3-op
    half-shift-averaged Schraudolph bf16 approximation, ~1% max err) so the
    Scalar engine stops being the sole critical path.
  * AV uses V as the stationary operand producing A^T[33, 1024] per head in
    PSUM (row 32 = softmax denominators via the ones column). A^T goes
    PSUM -> SBUF bf16 (with the token axis permuted p-major) -> DRAM -> xbar
    transpose back to [token, c], where a reciprocal-multiply normalizes into
    an SBUF accumulator; one contiguous 1 MB DMA writes the final output.
  * Matmuls are emitted in homogeneous per-head runs (16 logits MMs, then the
    previous head's 16 AV MMs) to keep LDWEIGHTS pipelined and the PE warm.
"""

import numpy as np
import ml_dtypes

import concourse.bacc as bacc
import concourse.mybir as mybir
from concourse.tile import TileContext
from concourse.bass_utils import run_bass_kernel_spmd

F32 = mybir.dt.float32
BF16 = mybir.dt.bfloat16
I16 = mybir.dt.int16
AF = mybir.ActivationFunctionType
ALU = mybir.AluOpType

N_CORES = 8
N = 1024          # tokens per batch (32 x 32)
NH = 8            # heads
EXP_SCALE = float(1.0 / np.sqrt(32.0))

# Shift-averaged Schraudolph bf16 exp approximation (VectorE path):
#   b  = rint(x*TRICK_A + TRICK_B)   (int16)
#   y  = bf16(b) + bf16(b + 61)
# max rel err ~1.34e-2, rms ~7.1e-3 for exp(EXP_SCALE * x); the unweighted add
# keeps the combine on the fast 2-source tensor_tensor path.
TRICK_A = float(EXP_SCALE * np.log2(np.e) * 128.0)
TRICK_B = float(16256.0 - 128.0 - 39.5625)
TRICK_D = 61
TRICK_SET = (2, 5)          # m-chunks per head computed on VectorE

_CACHE = {}


def _emit(tc, qet, kei, vpi, rti, out):
    nc = tc.nc

    with tc.tile_pool(name="big", bufs=1) as big:

        qe = big.tile([96, NH * N], BF16, name="qe")
        ke = big.tile([96, NH * N], BF16, name="ke")
        vp = big.tile([128, 8 * NH * 33], BF16, name="vp")
        rt = big.tile([32, 128], BF16, name="rt")
        nc.sync.dma_start(out=rt[:], in_=rti[:, :])
        nc.sync.dma_start(out=qe[0:32, 0:4 * N], in_=qet[:, 0:4 * N])
        nc.scalar.dma_start(out=qe[0:32, 4 * N:8 * N], in_=qet[:, 4 * N:8 * N])
        nc.scalar.dma_start(out=ke[:], in_=kei[:, :])
        nc.scalar.dma_start(out=vp[:], in_=vpi[:, :])

        qe_r = qe[:].rearrange("p (h nx ny) -> p h nx ny", h=NH, nx=32)

        # ---- PE warm-up: dense junk matmuls while the input DMAs land, so the
        # HAM clock gate reaches 8/8 before the B phase (results unused).
        with tc.tile_pool(name="wup", bufs=1, space="PSUM") as wup:
            w_ps = wup.tile([128, 512], F32, name="w_ps")
            for _ in range(30):
                nc.tensor.matmul(
                    w_ps[:, 0:128], rt[:, 0:128], rt[:, :],
                    start=True, stop=True,
                )

        # ---- B rows of QE: one matmul per shift t covering all 8 heads
        # (w-term shifts with y(n), h-term with x(n)); groups of 8 shifts per
        # PSUM tile, then one strided copy each into qe rows 32-63 / 64-95.
        with tc.tile_pool(name="bpp", bufs=2, space="PSUM") as bpp:
            for g in range(4):
                b_ps = bpp.tile([64, 2048], F32, name="b_ps")
                for tt in range(8):
                    t = g * 8 + tt
                    nc.tensor.matmul(
                        b_ps[0:32, tt * 256:(tt + 1) * 256],
                        rt[0:32, 31 - t:63 - t],
                        qe_r[0:32, :, :, t:t + 1],
                        start=True, stop=True,
                        tile_position=(0, 0),
                    )
                    nc.tensor.matmul(
                        b_ps[32:64, tt * 256:(tt + 1) * 256],
                        rt[0:32, 95 - t:127 - t],
                        qe_r[0:32, :, t:t + 1, :],
                        start=True, stop=True,
                        tile_position=(0, 32),
                    )
                bw = b_ps[0:32, :].rearrange("p (y h nx) -> p h nx y", y=8, h=NH)
                bh = b_ps[32:64, :].rearrange("p (nx h y) -> p h nx y", nx=8, h=NH)
                if g < 3:
                    nc.scalar.copy(qe_r[32:64, :, :, g * 8:(g + 1) * 8], bw)
                    nc.vector.tensor_copy(
                        qe_r[64:96, :, g * 8:(g + 1) * 8, :], bh)
                else:
                    # last group gates the main loop: split each copy across
                    # both engines so it drains ~2x faster
                    nc.scalar.copy(qe_r[32:64, 0:4, :, g * 8:(g + 1) * 8],
                                   bw[:, 0:4])
                    nc.vector.tensor_copy(qe_r[32:64, 4:8, :, g * 8:(g + 1) * 8],
                                          bw[:, 4:8])
                    nc.scalar.copy(qe_r[64:96, 0:4, g * 8:(g + 1) * 8, :],
                                   bh[:, 0:4])
                    nc.vector.tensor_copy(qe_r[64:96, 4:8, g * 8:(g + 1) * 8, :],
                                          bh[:, 4:8])

        # ---- main loop
        out_pt = out.rearrange("(p t) c -> p (t c)", p=128)
        with tc.tile_pool(name="lpp", bufs=3, space="PSUM") as lpp, \
             tc.tile_pool(name="app", bufs=1, space="PSUM") as app, \
             tc.tile_pool(name="ptp", bufs=18) as ptp, \
             tc.tile_pool(name="tbp", bufs=3) as tbp, \
             tc.tile_pool(name="tb2p", bufs=3) as tb2p, \
             tc.tile_pool(name="asp", bufs=3) as asp, \
             tc.tile_pool(name="ttp", bufs=3) as ttp, \
             tc.tile_pool(name="rp", bufs=3) as rp:
            o_all = big.tile([128, 8 * NH * 32], F32, name="o_all")
            o_r = o_all[:].rearrange("p (t h c) -> p t h c", t=8, h=NH)

            heads = {}  # h -> (at_ps, pts)
            tails = {}  # h -> at_sb

            def logits_and_exp(h):
                at_ps = app.tile([33, N], F32, name="at_ps")
                pts = []
                for i in range(8):
                    l_ps = lpp.tile([128, N], F32, name="l_ps")
                    for c in range(2):
                        nc.tensor.matmul(
                            l_ps[:, c * 512:(c + 1) * 512],
                            ke[:, h * N + i * 128: h * N + i * 128 + 128],
                            qe[:, h * N + c * 512: h * N + (c + 1) * 512],
                            start=True, stop=True,
                        )
                    pt = ptp.tile([128, N], BF16, name="pt")
                    if i in TRICK_SET:
                        tb = tbp.tile([128, N], I16, name="tb")
                        tb2 = tb2p.tile([128, N], I16, name="tb2")
                        nc.vector.tensor_scalar(
                            tb[:], l_ps[:], TRICK_A, TRICK_B, ALU.mult, ALU.add,
                        )
                        nc.vector.tensor_scalar(tb2[:], tb[:], TRICK_D, None, ALU.add)
                        nc.vector.tensor_tensor(
                            pt[:], tb[:].bitcast(BF16), tb2[:].bitcast(BF16),
                            ALU.add,
                        )
                    else:
                        nc.scalar.activation(pt[:], l_ps[:], AF.Exp, scale=EXP_SCALE)
                    pts.append(pt)
                heads[h] = (at_ps, pts)

            def av_and_store(h):
                at_ps, pts = heads.pop(h)
                for i in range(8):
                    for c in range(2):
                        nc.tensor.matmul(
                            at_ps[:, c * 512:(c + 1) * 512],
                            vp[:, (i * NH + h) * 33:(i * NH + h) * 33 + 33],
                            pts[i][:, c * 512:(c + 1) * 512],
                            start=(i == 0), stop=(i == 7),
                        )
                # PSUM -> SBUF bf16 with token axis permuted so that after the
                # xbar transpose, partition p holds tokens p*8..p*8+7. The
                # tile is 48 rows for the xbar's 16-row granularity (33-47
                # stale, transposed into unread columns).
                at_sb = asp.tile([48, N], BF16, name="at_sb")
                eng = nc.scalar if h % 2 == 0 else nc.vector
                copy = eng.copy if h % 2 == 0 else eng.tensor_copy
                copy(
                    at_sb[0:33, :].rearrange("p (j q) -> p j q", j=8),
                    at_ps[:].rearrange("p (q j) -> p j q", q=128),
                )
                tails[h] = at_sb

            tlds = {}  # h -> at_t

            def head_tail_tld(h):
                # de-transpose head h via the SBUF->SBUF DMA xbar
                at_sb = tails.pop(h)
                at_t = ttp.tile([128, 8 * 48], BF16, name="at_t")
                nc.sync.dma_start(
                    out=at_t[:].rearrange("p (j r) -> p j r", j=8),
                    in_=at_sb[:], transpose=True,
                )
                tlds[h] = at_t

            def head_tail_norm(h):
                # deferred one head so the xbar transfer has landed before the
                # strict-FIFO Vector queue reaches the reciprocal
                at3 = tlds.pop(h)[:].rearrange("p (j r) -> p j r", j=8)
                rr = rp.tile([128, 8], F32, name="rr")
                rr_r = rr[:].rearrange("p (j o) -> p j o", j=8)
                nc.vector.reciprocal(rr_r, at3[:, :, 32:33])
                nc.vector.tensor_tensor(
                    o_r[:, :, h, :],
                    at3[:, :, 0:32],
                    rr_r.broadcast_to((128, 8, 32)),
                    ALU.mult,
                )

            for h in range(NH):
                logits_and_exp(h)
                if h > 0:
                    av_and_store(h - 1)
                    head_tail_tld(h - 1)
                if h > 1:
                    head_tail_norm(h - 2)
            av_and_store(NH - 1)
            head_tail_tld(NH - 1)
            head_tail_norm(NH - 2)
            head_tail_norm(NH - 1)
            # final 1 MB store split across both HWDGE queues (halves are
            # token-row ranges, contiguous 4 KB runs per partition)
            nc.scalar.dma_start(out=out_pt[:, 0:4 * 256], in_=o_all[:, 0:4 * 256])
            nc.sync.dma_start(out=out_pt[:, 4 * 256:8 * 256],
                              in_=o_all[:, 4 * 256:8 * 256])


def build_nc():
    if "nc" in _CACHE:
        return _CACHE["nc"]
    nc = bacc.Bacc(
        "TRN2", target_bir_lowering=False, debug=False, num_devices=N_CORES
    )
    qet = nc.dram_tensor("qet", [32, NH * N], BF16, kind="ExternalInput")
    kei = nc.dram_tensor("kei", [96, NH * N], BF16, kind="ExternalInput")
    vpi = nc.dram_tensor("vpi", [128, 8 * NH * 33], BF16, kind="ExternalInput")
    rti = nc.dram_tensor("rti", [32, 128], BF16, kind="ExternalInput")
    out = nc.dram_tensor("out", [N, 256], F32, kind="ExternalOutput")
    with TileContext(nc) as tc:
        _emit(tc, qet.ap(), kei.ap(), vpi.ap(), rti.ap(), out.ap())
    nc.compile()
    _CACHE["nc"] = nc
    return nc


def _marshal(inputs, key_rel_w, key_rel_h):
    """Host-side layout/dtype marshalling (no math beyond the transforms the
    reference applies to index/layout)."""
    bf = ml_dtypes.bfloat16
    B = inputs.shape[0]
    x = np.ascontiguousarray(inputs.reshape(B, N, 768), dtype=np.float32)

    # Q^T image: [32 d, (h, n)]
    qet = np.ascontiguousarray(
        x[:, :, 0:256].reshape(B, N, NH, 32).transpose(0, 3, 2, 1)
        .reshape(B, 32, NH * N).astype(bf))
    # K^T rows + one-hot selector rows: [96, (h, n)]
    kT = (x[:, :, 256:512].reshape(B, N, NH, 32).transpose(0, 3, 2, 1)
          .reshape(B, 32, NH * N))
    m = np.arange(N)
    aw = (np.arange(32)[:, None] == (m % 32)[None, :]).astype(np.float32)
    ah = (np.arange(32)[:, None] == (m // 32)[None, :]).astype(np.float32)
    oh = np.tile(np.concatenate([aw, ah], 0), (1, NH))       # [64, NH*N]
    kei = np.ascontiguousarray(
        np.concatenate([kT, np.broadcast_to(oh, (B, 64, NH * N))], 1).astype(bf))
    # V packed to SBUF layout with ones column: [128 p, (t, h, 33)]
    v = x[:, :, 512:768].reshape(B, 8, 128, NH, 32).transpose(0, 2, 1, 3, 4)
    vpi = np.concatenate([v, np.ones((B, 128, 8, NH, 1), np.float32)], -1)
    vpi = np.ascontiguousarray(vpi.reshape(B, 128, 8 * NH * 33).astype(bf))
    # rel tables transposed: [32 d, 128] (cols 0-62 w, 64-126 h, 63/127 zero)
    rti = np.zeros((32, 128), np.float32)
    rti[:, 0:63] = np.asarray(key_rel_w, np.float32).T
    rti[:, 64:127] = np.asarray(key_rel_h, np.float32).T
    rti = np.ascontiguousarray(rti.astype(bf))
    return qet, kei, vpi, rti


def kernel(inputs, key_rel_w, key_rel_h):
    assert inputs.shape == (8, 32, 32, 768), inputs.shape
    nc = build_nc()
    qet, kei, vpi, rti = _marshal(inputs, key_rel_w, key_rel_h)
    in_maps = [
        {"qet": qet[b], "kei": kei[b], "vpi": vpi[b], "rti": rti}
        for b in range(N_CORES)
    ]
    res = run_bass_kernel_spmd(nc, in_maps, list(range(N_CORES)))
    return np.stack(
        [res.results[b]["out"].reshape(32, 32, 256) for b in range(N_CORES)]
    )


if __name__ == "__main__":
    rng = np.random.default_rng(0)
    inputs = rng.standard_normal((8, 32, 32, 768), dtype=np.float32)
    rw = rng.standard_normal((63, 32), dtype=np.float32) * 32 ** -0.5
    rh = rng.standard_normal((63, 32), dtype=np.float32) * 32 ** -0.5
    o = kernel(inputs, rw, rh)
    print(o.shape, o.dtype, float(np.abs(o).max()))
